# revision 35
# baseline (speedup 1.0000x reference)
"""Multi-region RNN kernel for Trainium2 (8 NeuronCores, SPMD time-sharded).

Model (per step t):
    inp  = einsum('bi,rih->rbh', x_t, W_ih) + bias
    loc  = einsum('rbh,rhg->rbg', H, W_hh)
    msg  = einsum('ij,ibh->jbh', C, H)
    cross= einsum('rbh,rhg->rbg', msg, W_rhh)
    H'   = tanh(inp + loc + cross)
Output: stack H over t -> [T,B,R*H] @ W_out + b_out.

Distribution: the per-step cost is dominated by ~200 PE weight loads
(W_hh[r], W_rhh[r] per region), which is independent of batch size, while
the dynamics are strongly contracting (~0.65x/step, zero-state restart
converges to <3e-2 in 12 steps).  So instead of batch-parallelism we shard
TIME: core c computes the 16-step output window [16c, 16c+16) by running
the recurrence with the FULL batch (B=32) for L=28 steps from a zero
state starting at t=16c-12 (core 0 starts at t=0 exactly).  The 12-step
burn-in converges far below the bf16 noise floor (validated: rel err
0.004 vs 0.0039 for the batch-parallel baseline).  128 sequential steps
-> 28 per core.

Per core layout / step structure:
  state ring   [h=128, (slot=4, r=100, b=32)] bf16, r-major cols r*32+b.
  loc:   per region matmul Whh[r] (128x128 FWL weight load) x state slice
         [h, 32], accumulated in 7 region-chunk PSUM tiles (<=16 regions).
  msg:   one xbar DMA transpose of the state -> Hrm [(r,b)-part, h], then
         per 4-batch group: 4 matmuls (lhsT=Hrm[:,b,:] 128x128, rhs=C
         zero-padded to 128 rows) -> [h, (b,j)], evicted to Msg (j-major).
  cross: per region matmul Wrhh[r] x Msg slice, same PSUM chunk as loc.
  inp:   precomputed in 4-step blocks (Wih[r] loaded once per block, x
         moving [I, (t4,b32)]), evicted bf16 to SBUF; bias added via a
         SWDGE accumulate-DMA from a DRAM-resident broadcast bias image.
  tanh:  DVE add (psum + inp) then ACT tanh per chunk into the ring.
  out:   every 4 steps, project [h,(t4,b32)] ring slices against Wout[r]
         into a [128, 64] PSUM accumulator (all 28 steps projected; the
         host keeps rows [12:28), or [0:16) on core 0).
"""

import numpy as np
import ml_dtypes
from contextlib import ExitStack

import concourse.bass as bass
import concourse.bacc as bacc
import concourse.tile as tile
from concourse import mybir
from concourse.bass_utils import run_bass_kernel_spmd

T, B, I, H, R, O = 128, 32, 128, 128, 100, 64
NCORES = 8
WIN = 16                  # output window per core
L = 28                    # steps per core (12 burn-in + 16 window)
RB = R * B                # 3200 live state cols, r-major: col = r*B + b
RBP = 128 * B             # 4096 ring slot cols (regions padded to 128 so the
                          # xbar transpose writes all 128 hrm rows with zeros)
NSLOT = 4                 # state ring slots (= p3 block size)
NBLK = L // 4             # 7 input-drive blocks of 4 steps
# region chunks for loc/cross/tanh PSUM tiles (<=16 regions = 512 cols)
CH = [(r0, min(r0 + 16, R)) for r0 in range(0, R, 16)]

BF = mybir.dt.bfloat16
F32 = mybir.dt.float32
Act = mybir.ActivationFunctionType
ADD = mybir.AluOpType.add

# Optional walrus LDWEIGHTS pipelining (off by default in compile_bir_kernel).
import os as _os
if _os.environ.get("KERNEL_LDW_OPT", "0") == "1":
    import concourse.bass_utils as _bu
    if not getattr(_bu, "_ldw_opt_patched", False):
        _orig_run_command = _bu.run_command

        def _run_command_ldw(argv, **kwargs):
            argv = ["--enable-ldw-opt=true" if a == "--enable-ldw-opt=false" else a
                    for a in argv]
            return _orig_run_command(argv, **kwargs)

        _bu.run_command = _run_command_ldw
        _bu._ldw_opt_patched = True

_CACHE: dict = {}
NREP = 1   # test-only hook: repeat the whole body to measure device time deltas
DEBUG_DUMP = False  # dump final state ring to a "dbg" output


def _build_program():
    nc = bacc.Bacc(None, target_bir_lowering=False)

    xT_d = nc.dram_tensor("xT", [I, L * B], BF, kind="ExternalInput")      # [i,(t,b)]
    C_d = nc.dram_tensor("C", [128, R], BF, kind="ExternalInput")          # [i,j] zero-padded rows
    Whh_d = nc.dram_tensor("Whh", [H, R * H], BF, kind="ExternalInput")    # [h,(r,g)]
    Wrhh_d = nc.dram_tensor("Wrhh", [H, R * H], BF, kind="ExternalInput")
    Wih_d = nc.dram_tensor("Wih", [I, R * H], BF, kind="ExternalInput")
    Wout_d = nc.dram_tensor("Wout", [H, R * O], BF, kind="ExternalInput")  # [h,(r,o)]
    biasE_d = nc.dram_tensor("biasE", [H, RB], BF, kind="ExternalInput")   # bias[r,h] bcast over b
    bout_d = nc.dram_tensor("bout", [1, O], BF, kind="ExternalInput")
    ident_d = nc.dram_tensor("ident", [128, 128], BF, kind="ExternalInput")
    out_d = nc.dram_tensor("out", [L, B, O], F32, kind="ExternalOutput")
    dbg_d = (nc.dram_tensor("dbg", [H, NSLOT * RBP], BF, kind="ExternalOutput")
             if DEBUG_DUMP else None)

    with tile.TileContext(nc) as tc, ExitStack() as ctx:
        consts = ctx.enter_context(tc.tile_pool(name="consts", bufs=1))

        Whh_s = consts.tile([H, R * H], BF)
        nc.sync.dma_start(Whh_s[:], Whh_d[:])
        Wrhh_s = consts.tile([H, R * H], BF)
        nc.sync.dma_start(Wrhh_s[:], Wrhh_d[:])
        Wih_s = consts.tile([I, R * H], BF)
        nc.sync.dma_start(Wih_s[:], Wih_d[:])
        Wout_s = consts.tile([H, R * O], BF)
        nc.sync.dma_start(Wout_s[:], Wout_d[:])
        xT_s = consts.tile([I, L * B], BF)
        nc.sync.dma_start(xT_s[:], xT_d[:])
        C_s = consts.tile([128, R], BF)
        nc.sync.dma_start(C_s[:], C_d[:])
        bout_s = consts.tile([1, O], BF)
        nc.sync.dma_start(bout_s[:], bout_d[:])
        ident_s = consts.tile([128, 128], BF)
        nc.sync.dma_start(ident_s[:], ident_d[:])
        ones_s = consts.tile([1, H], BF)
        nc.vector.memset(ones_s[:], 1.0)

        def _emit_body(_rep, bctx):
            # state ring, slot-major with b-major slots: col = s*4096 + b*128
            # + r (r padded to 128).  The xbar DMA transpose is a per-128-col
            # -chunk transpose (out[p, chunk] = in[:, chunk*128 + p]), so a
            # b-major [h, 4096] slot transposes in ONE call into hrm[r, b, h].
            # Pad cols r=100..127 are zeroed once; the transpose refreshes all
            # 128 hrm rows every step (pads land as zeros, matching C's zero
            # pad rows) with no write-write hazards.  The output projection
            # uses the ring as the MOVING operand so its multi-slot access
            # pattern stays off the weight path.
            st_pool = bctx.enter_context(tc.tile_pool(name=f"st{_rep}", bufs=1))
            ring = st_pool.tile([H, NSLOT * RBP], BF, name=f"ring{_rep}")
            _CACHE["_dbg_ring"] = ring
            ringS = ring.rearrange("h (s c) -> h s c", s=NSLOT)
            ring4 = ring.rearrange("h (s b r) -> h s b r", s=NSLOT, b=B)
            nc.vector.memset(ring4[:, :, :, R:128], 0.0)
            hrm = st_pool.tile([128, B * H], BF, name=f"hrm{_rep}")
            hrm3 = hrm.rearrange("r (b h) -> r b h", h=H)
            msgT = st_pool.tile([H, RB], BF, name=f"msgT{_rep}")
            msgT3 = msgT.rearrange("h (j b) -> h j b", b=B)

            inp_pool = bctx.enter_context(tc.tile_pool(name=f"inp{_rep}", bufs=2))
            ot_pool = bctx.enter_context(tc.tile_pool(name=f"ot{_rep}", bufs=2))
            pa_ps = bctx.enter_context(tc.tile_pool(name=f"pa{_rep}", bufs=4, space="PSUM"))
            pm_ps = bctx.enter_context(tc.tile_pool(name=f"pm{_rep}", bufs=2, space="PSUM"))
            p1_ps = bctx.enter_context(tc.tile_pool(name=f"p1{_rep}", bufs=1, space="PSUM"))
            po_ps = bctx.enter_context(tc.tile_pool(name=f"po{_rep}", bufs=1, space="PSUM"))

            blks: dict = {}

            def p1_alloc(m):
                blk = inp_pool.tile([H, 4 * RB], BF, tag="inpblk")
                blks[m] = blk.rearrange("h (t r b) -> h t r b", t=4, b=B)

            def p1_bias(m):
                # prefill the block with the broadcast bias image; the
                # evictions then ADD the matmul result on top (explicit
                # read-after-write deps keep everything ordered)
                blk4 = blks[m]
                for tt in range(4):
                    nc.gpsimd.dma_start(out=blk4[:, tt, 0:R, :], in_=biasE_d[:])

            def p1_quads(m, quads):
                """Input-drive matmuls for block m (steps 4m..4m+3), 4 regions
                per PSUM tile; DVE eviction fuses the bias add in place."""
                blk4 = blks[m]
                for q in quads:
                    ps = p1_ps.tile([H, 512], F32, tag="p1ps")
                    for ri in range(4):
                        r = 4 * q + ri
                        nc.tensor.matmul(
                            ps[:, ri * 128:(ri + 1) * 128],
                            Wih_s[:, r * H:(r + 1) * H],
                            xT_s[:, m * 4 * B:(m + 1) * 4 * B],
                            start=(ri == 0), stop=(ri == 3),
                        )
                    # psum cols are (ri, t, b); view as (t, ri, b) to match blk
                    src = ps.rearrange("h (ri t b) -> h t ri b", ri=4, b=B)
                    dst = blk4[:, :, 4 * q:4 * q + 4, :]
                    nc.vector.tensor_tensor(dst, src, dst, ADD)

            # prologue: first two input blocks
            for m in (0, 1):
                if m >= NBLK:
                    continue
                p1_alloc(m)
                p1_bias(m)
                p1_quads(m, range(R // 4))

            for t in range(L):
                m = t // 4
                # software-pipelined input drive: block m+2 spread over steps
                # 4m..4m+3 (quads 0-6, 7-13, 14-20, 21-24)
                if m + 2 < NBLK and t % 4 == 0:
                    p1_alloc(m + 2)
                    p1_bias(m + 2)

                blk4 = blks[m]
                sp = (t - 1) % NSLOT
                sc = t % NSLOT

                # input-drive fill quads for the upcoming block, doled out
                # into PE bubbles (msg-phase gaps + step tail)
                fillq = []
                if m + 2 < NBLK:
                    q0f = (t % 4) * 7
                    fillq = list(range(q0f, min(q0f + 7, R // 4)))

                if t == 0:
                    # zero initial state: H_0 = tanh(inp_0)
                    for (r0, r1) in CH:
                        nc.scalar.activation(
                            out=ring4[:, 0, :, r0:r1].rearrange("h b r -> h r b"),
                            in_=blk4[:, 0, r0:r1, :], func=Act.Tanh)
                else:
                    prevC = ringS[:, sp, :]                      # [h, 4096]
                    prevB = ring4[:, sp, :, :]                   # [h, b, r]

                    # loc for the first 3 chunks fills the transpose window
                    pas = []
                    for (r0, r1) in CH[:3]:
                        pa = pa_ps.tile([H, 512], F32, tag="pa")
                        paR = pa.rearrange("h (r b) -> h r b", b=B)
                        for r in range(r0, r1):
                            nc.tensor.matmul(
                                paR[:, r - r0, :],
                                Whh_s[:, r * H:(r + 1) * H],
                                prevB[:, :, r],
                                start=(r == r0), stop=False)
                        pas.append(pa)

                    # state -> region-major: chunked xbar transposes (8
                    # batches each) so the first msg matmuls start ~3us
                    # earlier than a monolithic transpose would allow.  loc
                    # chunk 3 and input-drive quads fill the PE while it
                    # would otherwise idle at transpose-rate.
                    def loc_chunk(i):
                        r0, r1 = CH[i]
                        pa = pa_ps.tile([H, 512], F32, tag="pa")
                        paR = pa.rearrange("h (r b) -> h r b", b=B)
                        for r in range(r0, r1):
                            nc.tensor.matmul(
                                paR[:, r - r0, :],
                                Whh_s[:, r * H:(r + 1) * H],
                                prevB[:, :, r],
                                start=(r == r0), stop=False)
                        pas.append(pa)

                    for k in range(4):
                        nc.sync.dma_start(
                            out=hrm3[:, 8 * k:8 * (k + 1), :],
                            in_=prevC[:, 1024 * k:1024 * (k + 1)],
                            transpose=True)
                        for g in (2 * k, 2 * k + 1):
                            pm = pm_ps.tile([H, 4 * R], F32, tag="pm")
                            for bi in range(4):
                                b = 4 * g + bi
                                nc.tensor.matmul(
                                    pm[:, bi * R:(bi + 1) * R],
                                    hrm3[:, b, :], C_s[:],
                                    start=(bi == 0), stop=(bi == 3))
                            srcv = pm.rearrange("h (bi j) -> h bi j", bi=4)
                            dst = msgT3[:, :, 4 * g:4 * g + 4].rearrange("h j b -> h b j")
                            nc.scalar.copy(dst, srcv)
                        if k == 0:
                            loc_chunk(3)
                        elif fillq:
                            p1_quads(m + 2, fillq[:2])
                            del fillq[:2]

                    # cross + add + tanh per chunk; remaining loc chunks slot
                    # in as PSUM buffers free up
                    for i, (r0, r1) in enumerate(CH):
                        pa = pas[i]
                        paR = pa.rearrange("h (r b) -> h r b", b=B)
                        w = (r1 - r0) * B
                        # fold the staged input drive into the accumulator on
                        # the PE (identity stationary, inp moving) so the
                        # chunk chain is PE -> ACT with no DVE hop
                        nc.tensor.matmul(
                            pa[:, :w], ident_s[:],
                            blk4[:, t % 4, r0:r1, :],
                            start=False, stop=False)
                        for r in range(r0, r1):
                            nc.tensor.matmul(
                                paR[:, r - r0, :],
                                Wrhh_s[:, r * H:(r + 1) * H],
                                msgT[:, r * B:(r + 1) * B],
                                start=False, stop=(r == r1 - 1))
                        nc.scalar.activation(
                            out=ring4[:, sc, :, r0:r1].rearrange("h b r -> h r b"),
                            in_=pa.rearrange("h (r b) -> h r b", b=B)[:, :r1 - r0, :],
                            func=Act.Tanh)
                        # next loc chunk reuses this chunk's PSUM buffer; its
                        # readers (cross+tanh above) are now emitted, so the
                        # pool WAR tracking orders the reuse correctly
                        if i + 4 < len(CH):
                            loc_chunk(i + 4)

                # input-drive fill work rides the step tail, soaking up the
                # PE bubble while the last tanh chunks drain
                if fillq:
                    p1_quads(m + 2, list(fillq))

                # output projection every 4 steps (ring slots 0..3 = t-3..t)
                if t % 4 == 3:
                    # transposed projection: out[o, (s,b)] += Wout[r].T @ ring,
                    # so the multi-slot ring AP rides the moving operand and
                    # the weight AP (Wout slice) stays one-free-dim.
                    po = po_ps.tile([O, NSLOT * B], F32, tag="po")
                    for r in range(R):
                        nc.tensor.matmul(
                            po[:], Wout_s[:, r * O:(r + 1) * O],
                            ring4[:, :, :, r],
                            start=(r == 0), stop=False)
                    nc.tensor.matmul(po[:], bout_s[:], ones_s[:, 0:NSLOT * B],
                                     start=False, stop=True)
                    ot = ot_pool.tile([O, NSLOT * B], F32, tag="ot")
                    nc.vector.tensor_scalar_add(ot[:], po[:], 0.0)
                    nc.sync.dma_start(
                        out=out_d[t - 3:t + 1, :, :].rearrange("t b o -> o t b"),
                        in_=ot[:])

        for _rep in range(NREP):
            with ExitStack() as bctx:
                _emit_body(_rep, bctx)
        if DEBUG_DUMP:
            nc.sync.dma_start(out=dbg_d[:], in_=_CACHE["_dbg_ring"][:])

    nc.compile()
    return nc


def _prep_inputs(x, C, W_ih, W_hh, W_rhh, bias, W_out, b_out):
    bf = ml_dtypes.bfloat16
    Cpad = np.zeros((128, R), np.float32)
    Cpad[:R, :] = C
    biasE = np.repeat(bias.T[:, :, None], B, axis=2).reshape(H, RB)
    shared = {
        "C": Cpad.astype(bf),
        "Whh": np.ascontiguousarray(W_hh.transpose(1, 0, 2).reshape(H, R * H)).astype(bf),
        "Wrhh": np.ascontiguousarray(W_rhh.transpose(1, 0, 2).reshape(H, R * H)).astype(bf),
        "Wih": np.ascontiguousarray(W_ih.transpose(1, 0, 2).reshape(I, R * H)).astype(bf),
        "Wout": np.ascontiguousarray(
            W_out.reshape(R, H, O).transpose(1, 0, 2).reshape(H, R * O)
        ).astype(bf),
        "biasE": np.ascontiguousarray(biasE).astype(bf),
        "bout": np.ascontiguousarray(b_out.reshape(1, O)).astype(bf),
        "ident": np.eye(128, dtype=np.float32).astype(bf),
    }
    in_maps = []
    for c in range(NCORES):
        t_lo = 0 if c == 0 else 16 * c + WIN - L
        xc = x[t_lo:t_lo + L]                                # [L, B, I]
        xT = np.ascontiguousarray(xc.transpose(2, 0, 1).reshape(I, L * B)).astype(bf)
        m = dict(shared)
        m["xT"] = xT
        in_maps.append(m)
    return in_maps


def kernel(x, C, W_ih, W_hh, W_rhh, bias, W_out, b_out, _trace=False):
    x = np.asarray(x, np.float32)
    in_maps = _prep_inputs(
        x, np.asarray(C, np.float32), np.asarray(W_ih, np.float32),
        np.asarray(W_hh, np.float32), np.asarray(W_rhh, np.float32),
        np.asarray(bias, np.float32), np.asarray(W_out, np.float32),
        np.asarray(b_out, np.float32),
    )
    if "nc" not in _CACHE:
        _CACHE["nc"] = _build_program()
    nc = _CACHE["nc"]
    res = run_bass_kernel_spmd(nc, in_maps, list(range(NCORES)), trace=_trace)
    out = np.empty((T, B, O), np.float32)
    for c in range(NCORES):
        oc = res.results[c]["out"]                           # [L, B, O]
        if c == 0:
            out[0:WIN] = oc[0:WIN]
        else:
            out[16 * c:16 * c + WIN] = oc[L - WIN:L]
    if _trace:
        return out, res
    return out


# revision 37
# speedup vs baseline: 1.2506x; 1.2506x over previous
"""Multi-region RNN kernel for Trainium2 (8 NeuronCores, SPMD time-sharded).

Model (per step t):
    inp  = einsum('bi,rih->rbh', x_t, W_ih) + bias
    loc  = einsum('rbh,rhg->rbg', H, W_hh)
    msg  = einsum('ij,ibh->jbh', C, H)
    cross= einsum('rbh,rhg->rbg', msg, W_rhh)
    H'   = tanh(inp + loc + cross)
Output: stack H over t -> [T,B,R*H] @ W_out + b_out.

Distribution: the per-step cost is dominated by ~200 PE weight loads
(W_hh[r], W_rhh[r] per region), which is independent of batch size, while
the dynamics are strongly contracting (~0.65x/step, zero-state restart
converges to <3e-2 in 12 steps).  So instead of batch-parallelism we shard
TIME: core c computes the 16-step output window [16c, 16c+16) by running
the recurrence with the FULL batch (B=32) for L=28 steps from a zero
state starting at t=16c-12 (core 0 starts at t=0 exactly).  The 12-step
burn-in converges far below the bf16 noise floor (validated: rel err
0.004 vs 0.0039 for the batch-parallel baseline).  128 sequential steps
-> 28 per core.

Per core layout / step structure:
  state ring   [h=128, (slot=4, r=100, b=32)] bf16, r-major cols r*32+b.
  loc:   per region matmul Whh[r] (128x128 FWL weight load) x state slice
         [h, 32], accumulated in 7 region-chunk PSUM tiles (<=16 regions).
  msg:   one xbar DMA transpose of the state -> Hrm [(r,b)-part, h], then
         per 4-batch group: 4 matmuls (lhsT=Hrm[:,b,:] 128x128, rhs=C
         zero-padded to 128 rows) -> [h, (b,j)], evicted to Msg (j-major).
  cross: per region matmul Wrhh[r] x Msg slice, same PSUM chunk as loc.
  inp:   precomputed in 4-step blocks (Wih[r] loaded once per block, x
         moving [I, (t4,b32)]), evicted bf16 to SBUF; bias added via a
         SWDGE accumulate-DMA from a DRAM-resident broadcast bias image.
  tanh:  DVE add (psum + inp) then ACT tanh per chunk into the ring.
  out:   every 4 steps, project [h,(t4,b32)] ring slices against Wout[r]
         into a [128, 64] PSUM accumulator (all 28 steps projected; the
         host keeps rows [12:28), or [0:16) on core 0).
"""

import numpy as np
import ml_dtypes
from contextlib import ExitStack

import concourse.bass as bass
import concourse.bacc as bacc
import concourse.tile as tile
from concourse import mybir
from concourse.bass_utils import run_bass_kernel_spmd

T, B, I, H, R, O = 128, 32, 128, 128, 100, 64
NCORES = 8
WIN = 16                  # output window per core
L = 28                    # steps per core (12 burn-in + 16 window)
RB = R * B                # 3200 live state cols, r-major: col = r*B + b
RBP = 128 * B             # 4096 ring slot cols (regions padded to 128 so the
                          # xbar transpose writes all 128 hrm rows with zeros)
NSLOT = 4                 # state ring slots (= p3 block size)
NBLK = L // 4             # 7 input-drive blocks of 4 steps
# region chunks for loc/cross/tanh PSUM tiles (<=16 regions = 512 cols)
CH = [(r0, min(r0 + 16, R)) for r0 in range(0, R, 16)]

BF = mybir.dt.bfloat16
F32 = mybir.dt.float32
Act = mybir.ActivationFunctionType
ADD = mybir.AluOpType.add

_CACHE: dict = {}
NREP = 1   # test-only hook: repeat the whole body to measure device time deltas
DEBUG_DUMP = False  # dump final state ring to a "dbg" output


def _build_program():
    nc = bacc.Bacc(None, target_bir_lowering=False)

    xT_d = nc.dram_tensor("xT", [I, L * B], BF, kind="ExternalInput")      # [i,(t,b)]
    C_d = nc.dram_tensor("C", [128, R], BF, kind="ExternalInput")          # [i,j] zero-padded rows
    Whh_d = nc.dram_tensor("Whh", [H, R * H], BF, kind="ExternalInput")    # [h,(r,g)]
    Wrhh_d = nc.dram_tensor("Wrhh", [H, R * H], BF, kind="ExternalInput")
    Wih_d = nc.dram_tensor("Wih", [I, R * H], BF, kind="ExternalInput")
    Wout_d = nc.dram_tensor("Wout", [H, R * O], BF, kind="ExternalInput")  # [h,(r,o)]
    biasE_d = nc.dram_tensor("biasE", [H, RB], BF, kind="ExternalInput")   # bias[r,h] bcast over b
    bout_d = nc.dram_tensor("bout", [O, 1], F32, kind="ExternalInput")
    ident_d = nc.dram_tensor("ident", [128, 128], BF, kind="ExternalInput")
    out_d = nc.dram_tensor("out", [L, B, O], F32, kind="ExternalOutput")
    dbg_d = (nc.dram_tensor("dbg", [H, NSLOT * RBP], BF, kind="ExternalOutput")
             if DEBUG_DUMP else None)

    with tile.TileContext(nc) as tc, ExitStack() as ctx:
        consts = ctx.enter_context(tc.tile_pool(name="consts", bufs=1))

        Whh_s = consts.tile([H, R * H], BF)
        nc.sync.dma_start(Whh_s[:], Whh_d[:])
        Wrhh_s = consts.tile([H, R * H], BF)
        nc.sync.dma_start(Wrhh_s[:], Wrhh_d[:])
        Wih_s = consts.tile([I, R * H], BF)
        nc.sync.dma_start(Wih_s[:], Wih_d[:])
        Wout_s = consts.tile([H, R * O], BF)
        nc.sync.dma_start(Wout_s[:], Wout_d[:])
        xT_s = consts.tile([I, L * B], BF)
        nc.sync.dma_start(xT_s[:], xT_d[:])
        C_s = consts.tile([128, R], BF)
        nc.sync.dma_start(C_s[:], C_d[:])
        bout_s = consts.tile([O, 1], F32)
        nc.sync.dma_start(bout_s[:], bout_d[:])
        ident_s = consts.tile([128, 128], BF)
        nc.sync.dma_start(ident_s[:], ident_d[:])

        def _emit_body(_rep, bctx):
            # state ring, slot-major with b-major slots: col = s*4096 + b*128
            # + r (r padded to 128).  The xbar DMA transpose is a per-128-col
            # -chunk transpose (out[p, chunk] = in[:, chunk*128 + p]), so a
            # b-major [h, 4096] slot transposes in ONE call into hrm[r, b, h].
            # Pad cols r=100..127 are zeroed once; the transpose refreshes all
            # 128 hrm rows every step (pads land as zeros, matching C's zero
            # pad rows) with no write-write hazards.  The output projection
            # uses the ring as the MOVING operand so its multi-slot access
            # pattern stays off the weight path.
            st_pool = bctx.enter_context(tc.tile_pool(name=f"st{_rep}", bufs=1))
            ring = st_pool.tile([H, NSLOT * RBP], BF, name=f"ring{_rep}")
            _CACHE["_dbg_ring"] = ring
            ringS = ring.rearrange("h (s c) -> h s c", s=NSLOT)
            ring4 = ring.rearrange("h (s b r) -> h s b r", s=NSLOT, b=B)
            nc.vector.memset(ring4[:, :, :, R:128], 0.0)
            hrm = st_pool.tile([128, B * H], BF, name=f"hrm{_rep}")
            hrm3 = hrm.rearrange("r (b h) -> r b h", h=H)
            msgT = st_pool.tile([H, RB], BF, name=f"msgT{_rep}")
            msgT3 = msgT.rearrange("h (j b) -> h j b", b=B)

            inp_pool = bctx.enter_context(tc.tile_pool(name=f"inp{_rep}", bufs=2))
            ot_pool = bctx.enter_context(tc.tile_pool(name=f"ot{_rep}", bufs=2))
            pa_ps = bctx.enter_context(tc.tile_pool(name=f"pa{_rep}", bufs=4, space="PSUM"))
            pm_ps = bctx.enter_context(tc.tile_pool(name=f"pm{_rep}", bufs=2, space="PSUM"))
            p1_ps = bctx.enter_context(tc.tile_pool(name=f"p1{_rep}", bufs=1, space="PSUM"))
            po_ps = bctx.enter_context(tc.tile_pool(name=f"po{_rep}", bufs=1, space="PSUM"))

            blks: dict = {}

            def p1_alloc(m):
                blk = inp_pool.tile([H, 4 * RB], BF, tag="inpblk")
                blks[m] = blk.rearrange("h (t r b) -> h t r b", t=4, b=B)

            def p1_bias(m):
                # prefill the block with the broadcast bias image; the
                # evictions then ADD the matmul result on top (explicit
                # read-after-write deps keep everything ordered)
                blk4 = blks[m]
                for tt in range(4):
                    nc.gpsimd.dma_start(out=blk4[:, tt, 0:R, :], in_=biasE_d[:])

            def p1_quads(m, quads):
                """Input-drive matmuls for block m (steps 4m..4m+3), 4 regions
                per PSUM tile; DVE eviction fuses the bias add in place."""
                blk4 = blks[m]
                for q in quads:
                    ps = p1_ps.tile([H, 512], F32, tag="p1ps")
                    for ri in range(4):
                        r = 4 * q + ri
                        nc.tensor.matmul(
                            ps[:, ri * 128:(ri + 1) * 128],
                            Wih_s[:, r * H:(r + 1) * H],
                            xT_s[:, m * 4 * B:(m + 1) * 4 * B],
                            start=(ri == 0), stop=(ri == 3),
                        )
                    # psum cols are (ri, t, b); view as (t, ri, b) to match blk
                    src = ps.rearrange("h (ri t b) -> h t ri b", ri=4, b=B)
                    dst = blk4[:, :, 4 * q:4 * q + 4, :]
                    nc.vector.tensor_tensor(dst, src, dst, ADD)

            # prologue: first two input blocks
            for m in (0, 1):
                if m >= NBLK:
                    continue
                p1_alloc(m)
                p1_bias(m)
                p1_quads(m, range(R // 4))

            for t in range(L):
                m = t // 4
                # software-pipelined input drive: block m+2 spread over steps
                # 4m..4m+3 (quads 0-6, 7-13, 14-20, 21-24)
                if m + 2 < NBLK and t % 4 == 0:
                    p1_alloc(m + 2)
                    p1_bias(m + 2)

                blk4 = blks[m]
                sp = (t - 1) % NSLOT
                sc = t % NSLOT

                # input-drive fill quads for the upcoming block, doled out
                # into PE bubbles (msg-phase gaps + step tail)
                fillq = []
                if m + 2 < NBLK:
                    q0f = (t % 4) * 7
                    fillq = list(range(q0f, min(q0f + 7, R // 4)))

                if t == 0:
                    # zero initial state: H_0 = tanh(inp_0)
                    for (r0, r1) in CH:
                        nc.scalar.activation(
                            out=ring4[:, 0, :, r0:r1].rearrange("h b r -> h r b"),
                            in_=blk4[:, 0, r0:r1, :], func=Act.Tanh)
                else:
                    prevC = ringS[:, sp, :]                      # [h, 4096]
                    prevB = ring4[:, sp, :, :]                   # [h, b, r]

                    # loc for the first 3 chunks fills the transpose window
                    pas = []
                    for (r0, r1) in CH[:3]:
                        pa = pa_ps.tile([H, 512], F32, tag="pa")
                        paR = pa.rearrange("h (r b) -> h r b", b=B)
                        for r in range(r0, r1):
                            nc.tensor.matmul(
                                paR[:, r - r0, :],
                                Whh_s[:, r * H:(r + 1) * H],
                                prevB[:, :, r],
                                start=(r == r0), stop=False)
                        pas.append(pa)

                    # state -> region-major: chunked xbar transposes (8
                    # batches each) so the first msg matmuls start ~3us
                    # earlier than a monolithic transpose would allow.  loc
                    # chunk 3 and input-drive quads fill the PE while it
                    # would otherwise idle at transpose-rate.
                    def loc_chunk(i):
                        r0, r1 = CH[i]
                        pa = pa_ps.tile([H, 512], F32, tag="pa")
                        paR = pa.rearrange("h (r b) -> h r b", b=B)
                        for r in range(r0, r1):
                            nc.tensor.matmul(
                                paR[:, r - r0, :],
                                Whh_s[:, r * H:(r + 1) * H],
                                prevB[:, :, r],
                                start=(r == r0), stop=False)
                        pas.append(pa)

                    for k in range(4):
                        nc.sync.dma_start(
                            out=hrm3[:, 8 * k:8 * (k + 1), :],
                            in_=prevC[:, 1024 * k:1024 * (k + 1)],
                            transpose=True)
                        for g in (2 * k, 2 * k + 1):
                            pm = pm_ps.tile([H, 4 * R], F32, tag="pm")
                            for bi in range(4):
                                b = 4 * g + bi
                                nc.tensor.matmul(
                                    pm[:, bi * R:(bi + 1) * R],
                                    hrm3[:, b, :], C_s[:],
                                    start=(bi == 0), stop=(bi == 3))
                            srcv = pm.rearrange("h (bi j) -> h bi j", bi=4)
                            dst = msgT3[:, :, 4 * g:4 * g + 4].rearrange("h j b -> h b j")
                            nc.scalar.copy(dst, srcv)
                        if k == 0:
                            loc_chunk(3)
                        elif fillq:
                            p1_quads(m + 2, fillq[:2])
                            del fillq[:2]

                    # cross + add + tanh per chunk; remaining loc chunks slot
                    # in as PSUM buffers free up
                    for i, (r0, r1) in enumerate(CH):
                        pa = pas[i]
                        paR = pa.rearrange("h (r b) -> h r b", b=B)
                        w = (r1 - r0) * B
                        # fold the staged input drive into the accumulator on
                        # the PE (identity stationary, inp moving) so the
                        # chunk chain is PE -> ACT with no DVE hop
                        nc.tensor.matmul(
                            pa[:, :w], ident_s[:],
                            blk4[:, t % 4, r0:r1, :],
                            start=False, stop=False)
                        for r in range(r0, r1):
                            nc.tensor.matmul(
                                paR[:, r - r0, :],
                                Wrhh_s[:, r * H:(r + 1) * H],
                                msgT[:, r * B:(r + 1) * B],
                                start=False, stop=(r == r1 - 1))
                        nc.scalar.activation(
                            out=ring4[:, sc, :, r0:r1].rearrange("h b r -> h r b"),
                            in_=pa.rearrange("h (r b) -> h r b", b=B)[:, :r1 - r0, :],
                            func=Act.Tanh)
                        # next loc chunk reuses this chunk's PSUM buffer; its
                        # readers (cross+tanh above) are now emitted, so the
                        # pool WAR tracking orders the reuse correctly
                        if i + 4 < len(CH):
                            loc_chunk(i + 4)

                # input-drive fill work rides the step tail, soaking up the
                # PE bubble while the last tanh chunks drain
                if fillq:
                    p1_quads(m + 2, list(fillq))

                # output projection every 4 steps (ring slots 0..3 = t-3..t)
                if t % 4 == 3:
                    # transposed projection: out[o, (s,b)] += Wout[r].T @ ring,
                    # so the multi-slot ring AP rides the moving operand and
                    # the weight AP (Wout slice) stays one-free-dim.
                    po = po_ps.tile([O, NSLOT * B], F32, tag="po")
                    for r in range(R):
                        nc.tensor.matmul(
                            po[:], Wout_s[:, r * O:(r + 1) * O],
                            ring4[:, :, :, r],
                            start=(r == 0), stop=(r == R - 1))
                    ot = ot_pool.tile([O, NSLOT * B], F32, tag="ot")
                    nc.vector.tensor_scalar_add(ot[:], po[:], bout_s[:])
                    nc.sync.dma_start(
                        out=out_d[t - 3:t + 1, :, :].rearrange("t b o -> o t b"),
                        in_=ot[:])

        for _rep in range(NREP):
            with ExitStack() as bctx:
                _emit_body(_rep, bctx)
        if DEBUG_DUMP:
            nc.sync.dma_start(out=dbg_d[:], in_=_CACHE["_dbg_ring"][:])

    nc.compile()
    return nc


def _prep_inputs(x, C, W_ih, W_hh, W_rhh, bias, W_out, b_out):
    bf = ml_dtypes.bfloat16
    Cpad = np.zeros((128, R), np.float32)
    Cpad[:R, :] = C
    biasE = np.repeat(bias.T[:, :, None], B, axis=2).reshape(H, RB)
    shared = {
        "C": Cpad.astype(bf),
        "Whh": np.ascontiguousarray(W_hh.transpose(1, 0, 2).reshape(H, R * H)).astype(bf),
        "Wrhh": np.ascontiguousarray(W_rhh.transpose(1, 0, 2).reshape(H, R * H)).astype(bf),
        "Wih": np.ascontiguousarray(W_ih.transpose(1, 0, 2).reshape(I, R * H)).astype(bf),
        "Wout": np.ascontiguousarray(
            W_out.reshape(R, H, O).transpose(1, 0, 2).reshape(H, R * O)
        ).astype(bf),
        "biasE": np.ascontiguousarray(biasE).astype(bf),
        "bout": np.ascontiguousarray(b_out.reshape(O, 1)).astype(np.float32),
        "ident": np.eye(128, dtype=np.float32).astype(bf),
    }
    in_maps = []
    for c in range(NCORES):
        t_lo = 0 if c == 0 else 16 * c + WIN - L
        xc = x[t_lo:t_lo + L]                                # [L, B, I]
        xT = np.ascontiguousarray(xc.transpose(2, 0, 1).reshape(I, L * B)).astype(bf)
        m = dict(shared)
        m["xT"] = xT
        in_maps.append(m)
    return in_maps


def kernel(x, C, W_ih, W_hh, W_rhh, bias, W_out, b_out, _trace=False):
    x = np.asarray(x, np.float32)
    in_maps = _prep_inputs(
        x, np.asarray(C, np.float32), np.asarray(W_ih, np.float32),
        np.asarray(W_hh, np.float32), np.asarray(W_rhh, np.float32),
        np.asarray(bias, np.float32), np.asarray(W_out, np.float32),
        np.asarray(b_out, np.float32),
    )
    if "nc" not in _CACHE:
        _CACHE["nc"] = _build_program()
    nc = _CACHE["nc"]
    res = run_bass_kernel_spmd(nc, in_maps, list(range(NCORES)), trace=_trace)
    out = np.empty((T, B, O), np.float32)
    for c in range(NCORES):
        oc = res.results[c]["out"]                           # [L, B, O]
        if c == 0:
            out[0:WIN] = oc[0:WIN]
        else:
            out[16 * c:16 * c + WIN] = oc[L - WIN:L]
    if _trace:
        return out, res
    return out
